# revision 37
# baseline (speedup 1.0000x reference)
"""DCT heat-blur kernel for Trainium2 (8 NeuronCores, Bass/Tile).

Math: reference computes, per image X (one (batch, channel) slice):
    coefs = D X D^T;  coefs *= E;  out = D coefs D^T
with E[h,w] = exp(-(f_h^2 + f_w^2) t_b) = e e^T rank-1.  The elementwise
decay therefore factors through the transforms:
    out = (D diag(e) D) X (D diag(e) D)^T = W^T X W,   W = (D diag(e) D)^T.
W_b is a tiny per-batch 256x256 matrix; the device builds it from e_b
(256 floats/batch) and the DCT matrix:  W = (diag(e) D)^T D^T.
The device then does 2 GEMMs per image instead of 4 + an elementwise pass.

Device layout per 256x256 image: row-blocks a=0,1 of 128 rows each.
apply(A, R)[m,h] = sum_k A[k,m] R[k,h] = (A^T R)[m,h] via
matmul(out[mb], lhsT=A[:, a, mb*128:(mb+1)*128], rhs=R[:, a, :]) summed
over a.  out = apply(apply(X, W), W).

Matmuls run in fp32r (fp32 with 11-bit mantissa, full PE rate) unless
BASS_DCT_MM_DTYPE=float32. fp32r operands must come from "rounded"
producers, so host data is pre-rounded (RNE to 11-bit mantissa) and DMAd
into float32r-typed tiles; device-side producers write float32r outputs.

x / out are pre/post-permuted on host into the exact SBUF layout so all
big DMAs are fully contiguous (8KB per partition per transfer).

Sharding: pure data parallel over batch, 16 batches (48 images) per core.
"""

import os
import numpy as np

BATCH = 128
CHANNELS = 3
N = 256
N_CORES = 8
PB = BATCH // N_CORES          # batches per core
IMGS = PB * CHANNELS           # images per core
GRP = 4                        # images per DMA group (1 MiB transfers)
NG = IMGS // GRP               # groups per core

# set BASS_DCT_MM_DTYPE=float32 to fall back to exact-rate fp32 matmuls
_MM_DTYPE = os.environ.get("BASS_DCT_MM_DTYPE", "float16")

LAST_EXEC_TIME_NS = None
_NC_CACHE = {}


def _round_f32r(a):
    """Round fp32 array to fp32r (11-bit mantissa) with round-to-nearest-even."""
    u = np.ascontiguousarray(a, dtype=np.float32).view(np.uint32)
    bias = np.uint32(0x7FF) + ((u >> np.uint32(12)) & np.uint32(1))
    r = (u + bias) & np.uint32(0xFFFFF000)
    return r.view(np.float32)


def _install_ntff_hook():
    """Wire antenv.axon_hooks (missing in this image) so trace=True works."""
    import sys
    import types

    if "antenv.axon_hooks" in sys.modules:
        return
    try:
        import trn_agent_boot.trn_boot as tb

        hook = tb._ntff_profile_via_ctypes("/opt/axon/libaxon_pjrt.so")
    except Exception:
        hook = None
    m = types.ModuleType("antenv.axon_hooks")
    m.get_axon_ntff_profile_hook = lambda: hook
    m.set_axon_ntff_profile_hook = lambda h: None
    sys.modules["antenv.axon_hooks"] = m


def _build_nc():
    import concourse.bacc as bacc
    import concourse.tile as tile
    import concourse.mybir as mybir

    f32 = mybir.dt.float32
    mm_dt = getattr(mybir.dt, _MM_DTYPE)

    nc = bacc.Bacc("TRN2", target_bir_lowering=False, debug=False)
    # x/o are host-permuted: [group][partition][img_in_grp, rowblk, col]
    x_d = nc.dram_tensor("x", [NG, 128, GRP * 2 * N], mm_dt, kind="ExternalInput").ap()
    # w: host-built per-batch W matrices, [partition][batch, rowblk, col]
    w_d = nc.dram_tensor("w", [128, PB, 2, N], mm_dt, kind="ExternalInput").ap()
    o_d = nc.dram_tensor("o", [NG, 128, GRP * 2 * N], f32, kind="ExternalOutput").ap()

    PREFETCH = NG

    with tile.TileContext(nc) as tc:
        with (
            tc.tile_pool(name="const", bufs=1) as cpool,
            tc.tile_pool(name="apool", bufs=2) as apool,
            tc.tile_pool(name="xpool", bufs=PREFETCH + 1) as xpool,
            tc.tile_pool(name="tpool", bufs=6) as tpool,
            tc.tile_pool(name="opool", bufs=6) as opool,
            tc.tile_pool(name="ps1", bufs=4, space="PSUM") as ps1,
            tc.tile_pool(name="ps2", bufs=4, space="PSUM") as ps2,
        ):
            def ld_ring(g):
                return nc.sync

            # fp16 loads are tiny and fully prefetched on sync; late store
            # issues go via the sync engine/ring (idle once loads finish)
            # so they never wait behind ACT's copy work
            def st_ring(g):
                return nc.scalar if g < NG // 2 else nc.sync

            xt_tiles = {}

            def issue_load(g):
                xt = xpool.tile([128, GRP, 2, N], mm_dt)
                ld_ring(g).dma_start(
                    xt[:], x_d[g].rearrange("p (i a w) -> p i a w", i=GRP, a=2)
                )
                xt_tiles[g] = xt

            # host-built W in four INDEPENDENT quarter tiles so early
            # groups depend only on their own chunk's DMA (a single tile
            # would make the first matmul wait for all four chunk writes)
            w_q = []
            with tc.high_priority():
                wq0 = cpool.tile([128, 4, 2, N], mm_dt, name="wq0")
                nc.sync.dma_start(wq0[:], w_d[:, 0:4])
                w_q.append(wq0)
                # group 0 loaded in per-image pieces: tiny first transfers
                # finish almost immediately, so compute starts ~5us sooner
                xt0 = xpool.tile([128, GRP, 2, N], mm_dt)
                x0_src = x_d[0].rearrange("p (i a w) -> p i a w", i=GRP, a=2)
                for ii in range(GRP):
                    nc.sync.dma_start(xt0[:, ii], x0_src[:, ii])
                xt_tiles[0] = xt0
            for q in range(1, 4):
                wq = cpool.tile([128, 4, 2, N], mm_dt, name=f"wq{q}")
                nc.scalar.dma_start(wq[:], w_d[:, 4 * q : 4 * (q + 1)])
                w_q.append(wq)
            for g in range(1, PREFETCH):
                issue_load(g)

            for g in range(NG):
                if g + PREFETCH < NG:
                    issue_load(g + PREFETCH)
                xt = xt_tiles.pop(g)
                ot = opool.tile([128, GRP, 2, N], f32)
                for ii in range(GRP):
                    img = g * GRP + ii
                    b = img // CHANNELS
                    t1_ps = ps1.tile([128, 2, N], f32)
                    for mb in range(2):
                        for a in range(2):
                            nc.tensor.matmul(
                                t1_ps[:, mb, :],
                                lhsT=xt[:, ii, a, mb * 128 : (mb + 1) * 128],
                                rhs=w_q[b // 4][:, b % 4, a, :],
                                start=(a == 0),
                                stop=(a == 1),
                            )
                    t1_sb = tpool.tile([128, 2, N], mm_dt)
                    if ii % 2 == 0:
                        nc.vector.tensor_copy(out=t1_sb[:], in_=t1_ps[:])
                    else:
                        nc.scalar.copy(t1_sb[:], t1_ps[:])
                    t2_ps = ps2.tile([128, 2, N], f32)
                    for mb in range(2):
                        for a in range(2):
                            nc.tensor.matmul(
                                t2_ps[:, mb, :],
                                lhsT=t1_sb[:, a, mb * 128 : (mb + 1) * 128],
                                rhs=w_q[b // 4][:, b % 4, a, :],
                                start=(a == 0),
                                stop=(a == 1),
                            )
                    if ii % 2 == 0:
                        nc.scalar.copy(ot[:, ii], t2_ps[:])
                    else:
                        nc.vector.tensor_copy(out=ot[:, ii], in_=t2_ps[:])
                st_ring(g).dma_start(
                    o_d[g].rearrange("p (i a w) -> p i a w", i=GRP, a=2), ot[:]
                )

    nc.compile()
    return nc


def _get_nc():
    key = ("nc", _MM_DTYPE)
    if key not in _NC_CACHE:
        _NC_CACHE[key] = _build_nc()
    return _NC_CACHE[key]


def _host_w(blur_sigmas, fwd_steps):
    """Per-batch W_b = (D diag(e_b) D)^T in device layout [128, B_core-sliced]."""
    sig = np.asarray(blur_sigmas, dtype=np.float64)
    steps = np.asarray(fwd_steps).astype(np.int64)
    n = np.arange(N, dtype=np.float64)
    D = np.sqrt(2.0 / N) * np.cos(np.pi * (n[None, :] + 0.5) * n[:, None] / N)
    D[0] *= 1.0 / np.sqrt(2.0)
    freqs = np.pi * n / N
    np_dt = np.float16 if _MM_DTYPE == "float16" else np.float32
    uniq, inv = np.unique(steps, return_inverse=True)
    ms = np.empty((len(uniq), N, N), dtype=np_dt)
    for i, s in enumerate(uniq):
        t = sig[s] ** 2 / 2.0
        e = np.exp(-(freqs**2) * t)
        w = (D @ (e[:, None] * D)).T
        if _MM_DTYPE == "float32r":
            w = _round_f32r(w.astype(np.float32))
        ms[i] = w.astype(np_dt)
    w_all = ms[inv]  # [B, N, N]
    # device layout [128, B, 2, N]: [p, b, a, h] = W_b[a*128+p, h]
    return np.ascontiguousarray(
        w_all.reshape(BATCH, 2, 128, N).transpose(2, 0, 1, 3)
    )


def kernel(x, blur_sigmas, fwd_steps):
    global LAST_EXEC_TIME_NS
    from concourse import bass_utils

    x = np.ascontiguousarray(np.asarray(x), dtype=np.float32)
    assert x.shape == (BATCH, CHANNELS, N, N), x.shape
    if _MM_DTYPE == "float32r":
        x = _round_f32r(x)
    elif _MM_DTYPE == "float16":
        x = x.astype(np.float16)
    w_host = _host_w(blur_sigmas, fwd_steps)

    # device x layout: [core][NG, 128, GRP*2*N]
    # x[img, a*128+p, w] -> xc[g, p, (i, a, w)]
    xp = (
        x.reshape(N_CORES, NG, GRP, 2, 128, N)
        .transpose(0, 1, 4, 2, 3, 5)
        .reshape(N_CORES, NG, 128, GRP * 2 * N)
    )
    in_maps = []
    for i in range(N_CORES):
        in_maps.append(
            {
                "x": np.ascontiguousarray(xp[i]),
                "w": np.ascontiguousarray(w_host[:, i * PB : (i + 1) * PB]),
            }
        )

    nc = _get_nc()
    trace = os.environ.get("BASS_DCT_TRACE", "0") == "1"
    kwargs = {}
    if trace:
        _install_ntff_hook()
        kwargs["trace"] = True
        tmpdir = os.environ.get("BASS_DCT_TRACE_DIR")
        if tmpdir:
            kwargs["tmpdir"] = tmpdir
    res = None
    for attempt in range(3):
        try:
            res = bass_utils.run_bass_kernel_spmd(
                nc, in_maps, core_ids=list(range(N_CORES)), **kwargs
            )
            break
        except Exception:
            # transient NRT_EXEC_UNIT_UNRECOVERABLE has been observed on the
            # first execution of a freshly loaded NEFF; a retry succeeds
            if attempt == 2:
                raise
            import time as _time

            _time.sleep(2.0)
            kwargs.pop("trace", None)
            kwargs.pop("tmpdir", None)
    LAST_EXEC_TIME_NS = res.exec_time_ns

    # inverse permute: oc[g, p, (i, a, w)] -> out[img, a*128+p, w]
    oc = np.stack([res.results[i]["o"] for i in range(N_CORES)])
    out = (
        oc.reshape(N_CORES, NG, 128, GRP, 2, N)
        .transpose(0, 1, 3, 4, 2, 5)
        .reshape(BATCH, CHANNELS, N, N)
    )
    return np.ascontiguousarray(out)
